# revision 32
# baseline (speedup 1.0000x reference)
"""Cross-attention Trainium2 kernel (8 NeuronCores, batch-parallel).

Reference (per batch element b):
    q = x @ Wq.T ; k = y @ Wk.T ; v = y @ Wv.T          (heads = 8, head_dim = 96)
    S = q k^T * scale + relative_pos                     ([h, n, m])
    out = softmax(S, -1) @ v ; out = out @ Wp.T + bp

Strategy (v7 — v3 core with head/tail/queue fixes):
  - one batch element per NeuronCore (B == 8 == n_cores), no collectives
  - all matmul operands fp16; rel bias added via identity matmul into the
    S psum (keeps the PE dense so the DVFS p-state stays high)
  - K/Q/V computed upfront k-outer so each arriving DMA chunk feeds 8
    matmuls; critical chunk-0 inputs split across two queues each
  - attention software-pipelined depth-3: stb psum pool bufs=3, es pool
    bufs=4; PE order qk(0) qk(1) qk(2) av(0) qk(3) av(1) ...
  - V carries a trailing ones column (slot 96) so the softmax denominator
    is psum row 96 of the av output; normalization (den DMA-shift, recip,
    partition broadcast, multiply) is emitted DEFERRED one head late so
    the in-order DVE/GpSimd queues never wait on the den DMA round trip
  - rel streams whole fp16 head tiles through a 3-deep ring alternating
    the sync/gpsimd queues
  - proj is jj-major (full 768-col rows per 2-row group) with fp16
    output staged + DMA'd per group on sync/scalar: ~2us tail vs ~15us
"""

import os
import numpy as np
from contextlib import ExitStack

import concourse.bass as bass
import concourse.mybir as mybir
import concourse.tile as tile
from concourse import bacc
from concourse.bass_utils import run_bass_kernel_spmd

B, N, C = 8, 1024, 768
H, HD = 8, 96
KCH = C // 128     # 6 contraction chunks
NCH = N // 128     # 8 sequence chunks
RBUF = 4           # rel ring depth (head-size tiles)
SCALE = HD ** -0.5
F32 = mybir.dt.float32
F16 = mybir.dt.float16
MUL = mybir.AluOpType.mult
EXP = mybir.ActivationFunctionType.Exp

_CACHE = {}


def build_bass():
    if "nc" in _CACHE:
        return _CACHE["nc"]
    nc = bacc.Bacc("TRN2", target_bir_lowering=False, debug=False, num_devices=B)

    xt_d = nc.dram_tensor("xt", [128, KCH, N], F16, kind="ExternalInput").ap()
    yt_d = nc.dram_tensor("yt", [128, KCH, N], F16, kind="ExternalInput").ap()
    wq_d = nc.dram_tensor("wq", [128, KCH, C], F16, kind="ExternalInput").ap()
    wk_d = nc.dram_tensor("wk", [128, KCH, C], F16, kind="ExternalInput").ap()
    wv_d = nc.dram_tensor("wv", [128, KCH, C], F16, kind="ExternalInput").ap()
    wp_d = nc.dram_tensor("wp", [HD + 1, H, C], F16, kind="ExternalInput").ap()
    # rel.T arranged [h, p(128), mc(8), n(1024)]
    rel_d = nc.dram_tensor("rel", [H, 128, NCH, N], F16, kind="ExternalInput").ap()
    ones_d = nc.dram_tensor("onesv", [128, 1, 1], F16, kind="ExternalInput").ap()
    ident_d = nc.dram_tensor("ident", [128, 128], F16, kind="ExternalInput").ap()
    # out rows stored (p, jj) fp16: host transposes back to [jj*128+p, c]
    out_d = nc.dram_tensor("out", [128, NCH, C], F16, kind="ExternalOutput").ap()

    with tile.TileContext(nc) as tc:
        with ExitStack() as ctx:
            # psum: 3-buffer main rotation (6 banks) + 1 aux (2 banks)
            ps3 = ctx.enter_context(tc.tile_pool(name="ps3", bufs=3, space="PSUM"))
            ps1 = ctx.enter_context(tc.tile_pool(name="ps1", bufs=1, space="PSUM"))
            qk_pool = ctx.enter_context(tc.tile_pool(name="qk", bufs=2))
            vaug_pool = ctx.enter_context(tc.tile_pool(name="vaug", bufs=1))
            at_pool = ctx.enter_context(tc.tile_pool(name="at", bufs=1))
            rel_pool = ctx.enter_context(tc.tile_pool(name="rel", bufs=RBUF))
            wp_pool = ctx.enter_context(tc.tile_pool(name="wpp", bufs=1))
            ob_pool = ctx.enter_context(tc.tile_pool(name="ob", bufs=2))
            const_pool = ctx.enter_context(tc.tile_pool(name="const", bufs=1))

            ones_col = const_pool.tile([128, 1, 1], F16)
            nc.scalar.dma_start(ones_col[:], ones_d[:])
            id_sb = const_pool.tile([128, 128], F16)

            qth = qk_pool.tile([HD, H, N], F16, tag="qk", name="qth")
            kth = qk_pool.tile([HD, H, N], F16, tag="qk", name="kth")
            # V with ones in slot HD: vaug[m, mc, h, 96] = 1, [.., 0:96] = V
            vaug = vaug_pool.tile([128, NCH, H, HD + 1], F16)
            nc.vector.tensor_copy(vaug[:, :, :, HD],
                                  ones_col[:].to_broadcast([128, NCH, H]))
            # normalized attn out, head-major; row 96 of h=0 is ones so the
            # h=0 proj matmul adds bp as an extra contraction row
            at_hm = at_pool.tile([HD + 1, H, N], F16)
            nc.vector.tensor_copy(at_hm[HD:HD + 1, 0, :],
                                  ones_col[0:1, 0, :].to_broadcast([1, N]))
            wp_sb = wp_pool.tile([HD + 1, H, C], F16)

            # rel ring: RBUF head tiles [128, mc, n]
            rel_sb = [rel_pool.tile([128, NCH, N], F16, tag="rel", name=f"rel{i}")
                      for i in range(RBUF)]

            def rel_fetch(h, eng):
                eng.dma_start(rel_sb[h % RBUF][:], rel_d[h, :, :, :])

            def rel_fetch2(h, eng_a, eng_b):
                # split a head tile's mc halves across two queues
                t = rel_sb[h % RBUF]
                eng_a.dma_start(t[:, 0:NCH // 2, :], rel_d[h, :, 0:NCH // 2, :])
                eng_b.dma_start(t[:, NCH // 2:, :], rel_d[h, :, NCH // 2:, :])

            # ---------------- QKV projections ----------------
            with ExitStack() as qkv_ctx:
                w_pool = qkv_ctx.enter_context(tc.tile_pool(name="w", bufs=1))
                y_pool = qkv_ctx.enter_context(tc.tile_pool(name="y", bufs=1))

                # per-chunk tiles for the critical-path K inputs
                wk_c = [w_pool.tile([128, C], F16, name=f"wk{k}") for k in range(KCH)]
                yt_c = [y_pool.tile([128, N], F16, name=f"yt{k}") for k in range(KCH)]
                wq_sb = w_pool.tile([128, KCH, C], F16, name="wq")
                wv_sb = w_pool.tile([128, KCH, C], F16, name="wv")
                xt_c = [y_pool.tile([128, N], F16, name=f"xt{k}") for k in range(KCH)]

                # critical K inputs (wk/yt chunks, consumed at ~1.7us/chunk)
                # spread over all three DMA queues in consumption order
                nc.sync.dma_start(wk_c[0][:, 0:384], wk_d[:, 0, 0:384])
                nc.scalar.dma_start(wk_c[0][:, 384:768], wk_d[:, 0, 384:768])
                nc.gpsimd.dma_start(yt_c[0][:, 0:512], yt_d[:, 0, 0:512])
                nc.scalar.dma_start(yt_c[0][:, 512:1024], yt_d[:, 0, 512:1024])
                nc.sync.dma_start(wk_c[1][:], wk_d[:, 1, :])
                nc.gpsimd.dma_start(yt_c[1][:, 0:512], yt_d[:, 1, 0:512])
                nc.scalar.dma_start(yt_c[1][:, 512:1024], yt_d[:, 1, 512:1024])
                nc.sync.dma_start(wk_c[2][:], wk_d[:, 2, :])
                nc.scalar.dma_start(yt_c[2][:, 0:512], yt_d[:, 2, 0:512])
                nc.gpsimd.dma_start(yt_c[2][:, 512:1024], yt_d[:, 2, 512:1024])
                nc.sync.dma_start(wk_c[3][:], wk_d[:, 3, :])
                nc.gpsimd.dma_start(yt_c[3][:], yt_d[:, 3, :])
                nc.scalar.dma_start(wk_c[4][:], wk_d[:, 4, :])
                nc.sync.dma_start(yt_c[4][:], yt_d[:, 4, :])
                nc.scalar.dma_start(wk_c[5][:], wk_d[:, 5, :])
                nc.gpsimd.dma_start(yt_c[5][:], yt_d[:, 5, :])
                nc.sync.dma_start(id_sb[:], ident_d[:])
                for k in range(KCH):
                    nc.scalar.dma_start(xt_c[k][:], xt_d[:, k, :])
                nc.sync.dma_start(wq_sb[:], wq_d[:])
                nc.gpsimd.dma_start(wv_sb[:], wv_d[:])
                # rel ring prefill: heads 0-3; wp (needed only at proj) last
                rel_fetch(0, nc.sync)
                rel_fetch(1, nc.gpsimd)
                rel_fetch(2, nc.scalar)
                rel_fetch(3, nc.sync)
                nc.scalar.dma_start(wp_sb[:], wp_d[:])

                # K.T and Q.T head-major [HD, H, N]; SCALE folded into wq
                # host-side. k-outer across all 8 heads so each arriving
                # chunk feeds 8 matmuls vs the DMA cadence.
                for which in range(2):
                    for nb in range(2):
                        big = [ps3.tile([128, 1024], F32, tag="ps",
                                        name=f"qk_{which}_{nb}_{i}") for i in range(3)]
                        big.append(ps1.tile([128, 1024], F32, tag="ps1",
                                            name=f"qk_{which}_{nb}_3"))
                        pst = [big[i // 2][:HD, (i % 2) * 512:(i % 2 + 1) * 512]
                               for i in range(8)]
                        for k in range(KCH):
                            for h in range(H):
                                if which == 0:
                                    lhs = wk_c[k][:, h * HD:(h + 1) * HD]
                                    rhs = yt_c[k][:, nb * 512:(nb + 1) * 512]
                                else:
                                    lhs = wq_sb[:, k, h * HD:(h + 1) * HD]
                                    rhs = xt_c[k][:, nb * 512:(nb + 1) * 512]
                                nc.tensor.matmul(pst[h], lhs, rhs,
                                                 start=(k == 0),
                                                 stop=(k == KCH - 1))
                        dst = kth if which == 0 else qth
                        for h in range(H):
                            d_ap = dst[:, h, nb * 512:(nb + 1) * 512]
                            nc.vector.tensor_copy(d_ap, pst[h])

                # V: c-blocks aligned to head boundaries (5 heads | 3 heads)
                for c0, cw, h0, nh in ((0, 480, 0, 5), (480, 288, 5, 3)):
                    for mcp in range(4):          # mc pairs
                        pv = ps1.tile([128, 1024], F32, tag="ps1", name=f"v{c0}_{mcp}") \
                            if mcp == 3 else \
                            ps3.tile([128, 1024], F32, tag="ps", name=f"v{c0}_{mcp}")
                        psv = [pv[:, 0:cw], pv[:, 512:512 + cw]]
                        for k in range(KCH):
                            for i in range(2):
                                mc = mcp * 2 + i
                                nc.tensor.matmul(
                                    psv[i],
                                    yt_c[k][:, mc * 128:(mc + 1) * 128],
                                    wv_sb[:, k, c0:c0 + cw],
                                    start=(k == 0),
                                    stop=(k == KCH - 1),
                                )
                        for i in range(2):
                            mc = mcp * 2 + i
                            for hh in range(nh):
                                nc.vector.tensor_copy(
                                    vaug[:, mc, h0 + hh, 0:HD],
                                    psv[i][:, hh * HD:(hh + 1) * HD])

            # ---------------- attention ----------------
            with ExitStack() as att_ctx:
                es_pool = att_ctx.enter_context(tc.tile_pool(name="es", bufs=4))
                rc_pool = att_ctx.enter_context(tc.tile_pool(name="rc", bufs=2))
                au_pool = att_ctx.enter_context(tc.tile_pool(name="au", bufs=2))

                oabs = {}
                norm_state = {}

                for h in range(H):
                    if 1 <= h <= 4:   # fetch head h+3 into the slot head h-1 freed
                        if h % 2:
                            rel_fetch2(h + 3, nc.gpsimd, nc.sync)
                        else:
                            rel_fetch2(h + 3, nc.sync, nc.gpsimd)
                    relh = rel_sb[h % RBUF]

                    ess = [None] * NCH

                    def qk_stage(mc, h=h, relh=relh, ess=ess):
                        stb = ps3.tile([128, 1024], F32, tag="ps", name=f"st{h}_{mc}")
                        kt_sl = kth[:, h, mc * 128:(mc + 1) * 128]
                        es = es_pool.tile([128, N], F16, tag="es")
                        ess[mc] = es
                        for i in range(2):
                            sl = slice(i * 512, (i + 1) * 512)
                            nc.tensor.matmul(stb[:, sl], kt_sl, qth[:, h, sl],
                                             start=True, stop=False)
                        for i in range(2):
                            sl = slice(i * 512, (i + 1) * 512)
                            nc.tensor.matmul(stb[:, sl], id_sb[:], relh[:, mc, sl],
                                             start=False, stop=True)
                        nc.scalar.activation(es[:], stb[:], EXP)

                    def av_stage(mc, h=h, ess=ess):
                        if mc == 0:
                            oabs[h] = ps1.tile([128, 1024], F32, tag="ps1",
                                               name=f"oa_{h}")
                        oab = oabs[h]
                        va = vaug[:, mc, h, :]
                        es = ess[mc]
                        for i in range(2):
                            sl = slice(i * 512, (i + 1) * 512)
                            nc.tensor.matmul(oab[:HD + 1, sl], va, es[:, sl],
                                             start=(mc == 0), stop=(mc == NCH - 1))

                    def norm_a(h=h):
                        # right after av(h,7): au copies free the single oab
                        # buffer; den DMA-shifts row 96 -> partition 0
                        oab = oabs.pop(h)
                        au = au_pool.tile([HD + 1, N], F32, tag="au", name=f"au{h}")
                        nc.vector.tensor_copy(au[:, 0:512], oab[:HD + 1, 0:512])
                        nc.vector.tensor_copy(au[:, 512:1024],
                                              oab[:HD + 1, 512:1024])
                        den = rc_pool.tile([1, N], F32, tag="den", name=f"den{h}")
                        nc.sync.dma_start(den[:], au[HD:HD + 1, :])
                        norm_state[h] = (au, den)

                    def norm_b(hp):
                        au, den = norm_state[hp]
                        rcp = rc_pool.tile([1, N], F32, tag="rc", name=f"rc{hp}")
                        nc.vector.reciprocal_approx_fast(out=rcp[:], in_=den[:])
                        bcb = au_pool.tile([HD, N], F32, tag="bc", name=f"bc{hp}")
                        nc.gpsimd.partition_broadcast(bcb[:], rcp[0:1, :],
                                                      channels=HD)
                        norm_state[hp] = (au, bcb)

                    def norm_c(hp):
                        au, bcb = norm_state.pop(hp)
                        nc.vector.tensor_tensor(at_hm[0:HD, hp, 0:512],
                                                au[0:HD, 0:512],
                                                bcb[:, 0:512], MUL)
                        nc.vector.tensor_tensor(at_hm[0:HD, hp, 512:1024],
                                                au[0:HD, 512:1024],
                                                bcb[:, 512:1024], MUL)

                    qk_stage(0)
                    qk_stage(1)
                    qk_stage(2)
                    for mc in range(NCH):
                        av_stage(mc)
                        if mc == NCH - 1:
                            norm_a()
                        if mc == 1 and h > 0:
                            norm_b(h - 1)
                        if mc == 3 and h > 0:
                            norm_c(h - 1)
                        if mc + 3 < NCH:
                            qk_stage(mc + 3)
                norm_b(H - 1)
                norm_c(H - 1)

            # ---------------- output projection (jj-major) ----------------
            for jg in range(4):                  # groups of 2 row-chunks
                pa = ps3.tile([128, 1024], F32, tag="ps", name=f"pa{jg}")
                pb = (ps1 if jg % 2 else ps3).tile(
                    [128, 1024], F32, tag="ps1" if jg % 2 else "ps",
                    name=f"pb{jg}")
                # slices 512-spaced so each accumulation group owns a full
                # 2KB psum zero region
                po = [pa[:, 0:512], pa[:, 512:1024],
                      pb[:, 0:256], pb[:, 512:768]]
                for hh in range(H):
                    rows = HD + 1 if hh == 0 else HD
                    for j in range(4):
                        jj = jg * 2 + (j % 2)
                        c0, cw = (0, 512) if j < 2 else (512, 256)
                        nc.tensor.matmul(
                            po[j],
                            at_hm[:rows, hh, jj * 128:(jj + 1) * 128],
                            wp_sb[:rows, hh, c0:c0 + cw],
                            start=(hh == 0),
                            stop=(hh == H - 1),
                        )
                ot = ob_pool.tile([128, 2, C], F16, tag="ob", name=f"ot{jg}")
                for j in range(2):
                    nc.vector.tensor_copy(ot[:, j, 0:512], po[j])
                    nc.vector.tensor_copy(ot[:, j, 512:768], po[2 + j])
                # sync/scalar only: the gpsimd queue may still be draining rel
                (nc.scalar if jg % 2 else nc.sync).dma_start(
                    out_d[:, jg * 2:(jg + 1) * 2, :], ot[:])

    nc.compile()
    _CACHE["nc"] = nc
    return nc


def make_in_maps(x, y, relative_pos, Wq, Wk, Wv, Wp, bp):
    x = np.asarray(x, dtype=np.float32)
    y = np.asarray(y, dtype=np.float32)
    relative_pos = np.asarray(relative_pos, dtype=np.float32)
    Wq = np.asarray(Wq, dtype=np.float32)
    Wk = np.asarray(Wk, dtype=np.float32)
    Wv = np.asarray(Wv, dtype=np.float32)
    Wp = np.asarray(Wp, dtype=np.float32)
    bp = np.asarray(bp, dtype=np.float32)

    def wchunks(w):  # W.T [c_in, c_out] -> [128, KCH, C]
        return np.ascontiguousarray(
            w.T.reshape(KCH, 128, C).transpose(1, 0, 2)).astype(np.float16)

    wq_h = wchunks(Wq * SCALE)
    wk_h = wchunks(Wk)
    wv_h = wchunks(Wv)
    # Wp.T is [c'=(h,d), c]; head-major [d, h, c] + bias row at d=96
    wp_hm = Wp.T.reshape(H, HD, C).transpose(1, 0, 2)
    wp_h = np.zeros((HD + 1, H, C), dtype=np.float16)
    wp_h[:HD] = wp_hm.astype(np.float16)
    wp_h[HD, 0, :] = bp.astype(np.float16)
    # rel.T as [h, p, mc, n]: rel_h[h, p, mc, n] = rel[h, n, mc*128+p]
    relT = relative_pos.transpose(0, 2, 1)                  # [h, m, n]
    rel_h = np.ascontiguousarray(
        relT.reshape(H, NCH, 128, N).transpose(0, 2, 1, 3)).astype(np.float16)

    def tchunks(a):  # [n, c] -> a.T [c, n] -> [128, KCH, N]
        return np.ascontiguousarray(
            a.T.reshape(KCH, 128, N).transpose(1, 0, 2)).astype(np.float16)

    in_maps = []
    for b in range(B):
        in_maps.append({
            "xt": tchunks(x[b]),
            "yt": tchunks(y[b]),
            "wq": wq_h, "wk": wk_h, "wv": wv_h, "wp": wp_h,
            "rel": rel_h,
            "onesv": np.ones((128, 1, 1), dtype=np.float16),
            "ident": np.eye(128, dtype=np.float16),
        })
    return in_maps


def kernel(x, y, relative_pos, H=None, W=None, Wq=None, Wk=None, Wv=None, Wp=None, bp=None,
           **extra):
    nc = build_bass()
    in_maps = make_in_maps(x, y, relative_pos, Wq, Wk, Wv, Wp, bp)
    res = run_bass_kernel_spmd(nc, in_maps, list(range(B)))
    outs = []
    for b in range(B):
        o = res.results[b]["out"]                    # [128, NCH, C], rows (p, jj)
        outs.append(np.ascontiguousarray(
            o.transpose(1, 0, 2)).reshape(N, C).astype(np.float32))
    return np.stack(outs, axis=0)
